# revision 55
# baseline (speedup 1.0000x reference)
"""ArcFace loss kernel v4: classes-on-partitions, PE-reduced e5m2 exp dumps.

~82-85us on 8 cores (prior baseline 110us).  Data-parallel over rows N
(each core owns 1024 rows).  Differences vs the rows-on-partitions family:
the matmul puts CLASSES on PSUM partitions (lhsT = W chunk, rhs = x rows),
the drains are single-pass (ACT exp -> fp8e5 dump with no accum_out read;
DVE e5m2-Schraudolph int8 with no reduce tree), and the per-row exp-sum
reduction over classes is done by the PE via fp8e5 DoubleRow ones-matmuls
accumulating into a per-row-tile PSUM register (partition-dim reduction is
free on the PE; the drains' free-dim direction would have needed a second
elementwise pass).  The psum drain bandwidth (ACT 1.2GHz + DVE 0.96GHz,
1 elem/cycle/lane each, ~1.9 elem/ns aggregate) is the fundamental wall;
the schedule keeps both drain engines and the (mid-clock) PE co-saturated:
  - 80 units of [128 classes, 2x512 rows] on a 3-unit/6-bank psum ring,
    strictly alternating ACT/DVE drains (unit 79 forced to ACT so the 240
    W pads contribute exactly exp(0)=1 each; the tail subtracts 240);
  - PE ones-reduces run 3+ units behind their drains so the PE instruction
    stream never blocks mid-streak (p-state stays up);
  - drains are 1024 wide (2 banks): wider instructions amortize better but
    starve the PE on a 6-bank ring.

  - W is zero-padded to 10240 classes (80 chunks of 128).  The 240 pads sit
    in the last pair, which is always ACT-drained, so each contributes
    exp(0) = 1.0 exactly; the tail subtracts 240.
  - e5m2 dumps quantize exp to 2 mantissa bits: ~5.5% rms per-element noise
    that averages out over the 10240-term row sums (constants tuned so the
    expected sum is exact; measured end error ~1e-4).
  - Target path unchanged: host supplies t_s = S*<x_n, W[tgt]> exactly; the
    device computes the margin numerator with a Quake-rsqrt chain and
    ln(denom) with a bits->log2 approximation, all on DVE.
"""

import math

import numpy as np

S = 30.0
MARGIN = 0.3
EPS = 1e-7
N, D, C = 8192, 256, 10000
NCORES = 8
NLOC = N // NCORES  # 1024 rows per core
NJ = NLOC // 128
CP = 10240  # padded class count: 80 chunks of 128
NCHUNK = CP // 128  # 80
NPAIR = NCHUNK // 2  # 40 reduce pairs
NPAD = CP - C  # 240 pad classes, exp(0)=1 each (ACT-drained)
RT = 512  # row-tile width (rhs free per matmul)
NRT = NLOC // RT  # 2 row-tiles

SA = 8.0
SB = 3.75  # SA*SB = S

# f32 Schraudolph (target-score path)
AEXP = 12102203.0
BEXP = 1064881816.0
# e5m2 Schraudolph: int8(A5*x + B5) bitcast to fp8e5 ~= exp(x)
A5 = 5.770780163555856  # 4/ln2
B5 = 59.777
RSQRT_MAGIC = 1597463007.0

# unit seq = pair*2 + rowtile; engine plan: ~52% ACT / 48% DVE, with the
# pad-carrying last pair (units 78, 79) forced to ACT
_PLAN = [
    "A" if (s % 2 == 0 or s in (19, 59, 79)) else "D" for s in range(2 * NPAIR)
]

_CACHE = {}


def _build():
    import concourse.bass as bass  # noqa: F401
    import concourse.mybir as mybir
    import concourse.tile as tile
    from concourse import bacc

    f32 = mybir.dt.float32
    f8 = mybir.dt.float8e4
    f8e5 = mybir.dt.float8e5
    i8 = mybir.dt.int8
    i32 = mybir.dt.int32
    AF = mybir.ActivationFunctionType
    OP = mybir.AluOpType
    DR = mybir.MatmulPerfMode.DoubleRow

    nc = bacc.Bacc()
    xT_ext = nc.declare_dram_parameter("xT", [128, 2, NLOC], f8, isOutput=False)
    wt_ext = nc.declare_dram_parameter("wt", [128, 2, CP], f8, isOutput=False)
    ts_ext = nc.declare_dram_parameter("ts", [128, NJ], f32, isOutput=False)
    out_ext = nc.declare_dram_parameter("out", [1, 1], f32, isOutput=True)

    with tile.TileContext(nc) as tc:
        with (
            tc.tile_pool(name="singles", bufs=1) as singles,
            tc.tile_pool(name="dpool", bufs=12) as dpool,
            tc.tile_pool(name="pmain", bufs=1, space="PSUM") as psum_main,
        ):
            # banks 0-5: fill ring (3 units of 2 banks); banks 6/7: the two
            # per-row-tile sum accumulators (sums duplicated on all partitions)
            pm = psum_main.tile([128, 8 * 512], f32)
            SACCS = (7 * 512, 6 * 512)

            xT = singles.tile([128, 2, NLOC], f8)
            wt = singles.tile([128, 2, CP], f8)
            traw = singles.tile([128, NJ], f32)
            nc.scalar.dma_start(out=xT[:, :, 0:RT], in_=xT_ext[:, :, 0:RT])
            nc.scalar.dma_start(out=xT[:, :, RT:NLOC], in_=xT_ext[:, :, RT:NLOC])
            W_ROUNDS = [(0, 256), (256, 512), (512, 1024), (1024, 2048),
                        (2048, 4096), (4096, 6144), (6144, 8192), (8192, CP)]
            for r, (c0, c1) in enumerate(W_ROUNDS):
                eng = nc.sync if r % 2 == 0 else nc.scalar
                eng.dma_start(out=wt[:, :, c0:c1], in_=wt_ext[:, :, c0:c1])
            nc.scalar.dma_start(out=traw, in_=ts_ext[:, :])

            ones8 = singles.tile([128, 2, 128], f8e5)
            nc.vector.memset(ones8, 1.0)
            # preload the ACT Exp table while the W DMA streams (else the
            # first real drain pays the 1.3us ACT_TABLE_LOAD)
            tdump = singles.tile([128, 32], f32)
            nc.scalar.activation(
                out=tdump, in_=ones8.bitcast(f32)[:, 0:1], func=AF.Exp
            )

            rs_seed = singles.tile([128, NJ], i32)
            rs_t1 = singles.tile([128, NJ], f32)
            rs_y1 = singles.tile([128, NJ], f32)
            rs_t2 = singles.tile([128, NJ], f32)

            def rsqrt2(src, dst, fold=1.0):
                nc.vector.tensor_scalar(
                    out=rs_seed, in0=src.bitcast(i32), scalar1=-0.5,
                    scalar2=RSQRT_MAGIC, op0=OP.mult, op1=OP.add,
                )
                y0 = rs_seed.bitcast(f32)
                nc.vector.tensor_tensor(out=rs_t1, in0=y0, in1=y0, op=OP.mult)
                nc.vector.tensor_tensor(out=rs_t1, in0=rs_t1, in1=src, op=OP.mult)
                nc.vector.tensor_scalar(
                    out=rs_t1, in0=rs_t1, scalar1=-0.5, scalar2=1.5,
                    op0=OP.mult, op1=OP.add,
                )
                nc.vector.tensor_tensor(out=rs_y1, in0=y0, in1=rs_t1, op=OP.mult)
                nc.vector.tensor_tensor(out=rs_t2, in0=rs_y1, in1=rs_y1, op=OP.mult)
                nc.vector.tensor_tensor(out=rs_t2, in0=rs_t2, in1=src, op=OP.mult)
                nc.vector.tensor_scalar(
                    out=rs_t2, in0=rs_t2, scalar1=-0.5 * fold, scalar2=1.5 * fold,
                    op0=OP.mult, op1=OP.add,
                )
                nc.vector.tensor_tensor(out=dst, in0=rs_y1, in1=rs_t2, op=OP.mult)

            tcl = singles.tile([128, NJ], f32)
            usq = singles.tile([128, NJ], f32)
            rsu = singles.tile([128, NJ], f32)
            rtm = singles.tile([128, NJ], f32)
            numer = singles.tile([128, NJ], f32)
            exp_num = singles.tile([128, NJ], f32)
            exp_st = singles.tile([128, NJ], f32)

            def numer_chain():
                sclip = S * (1.0 - EPS)
                nc.vector.tensor_scalar(
                    out=tcl, in0=traw, scalar1=-sclip, scalar2=sclip,
                    op0=OP.max, op1=OP.min,
                )
                nc.vector.tensor_tensor(out=usq, in0=tcl, in1=tcl, op=OP.mult)
                nc.vector.tensor_scalar(
                    out=usq, in0=usq, scalar1=-1.0, scalar2=S * S,
                    op0=OP.mult, op1=OP.add,
                )
                rsqrt2(usq, rsu, fold=-math.sin(MARGIN))
                nc.vector.tensor_tensor(out=rtm, in0=usq, in1=rsu, op=OP.mult)
                nc.vector.scalar_tensor_tensor(
                    out=numer, in0=tcl, scalar=math.cos(MARGIN), in1=rtm,
                    op0=OP.mult, op1=OP.add,
                )

            # ---------------- main loop: 80 units of [128cls, 2, 512rows] --
            pend = []  # (dump, rowtile, pair) awaiting their PE reduce

            def flush_reduces():
                while pend:
                    dmp, r, p = pend.pop(0)
                    sc = SACCS[r]
                    nc.tensor.matmul(
                        out=pm[:, sc : sc + 512],
                        lhsT=ones8,
                        rhs=dmp.bitcast(f8e5),
                        start=(p == 0),
                        stop=(p == NPAIR - 1),
                        perf_mode=DR,
                        skip_group_check=True,
                    )

            for p in range(NPAIR):
                for r in range(NRT):
                    seq = p * 2 + r
                    b0 = (seq % 3) * 1024
                    ngg = 1 if p == NPAIR - 1 else 2
                    for gg in range(ngg):
                        g = 2 * p + gg
                        nc.tensor.matmul(
                            out=pm[:, b0 + gg * 512 : b0 + (gg + 1) * 512],
                            lhsT=wt[:, :, g * 128 : (g + 1) * 128],
                            rhs=xT[:, :, r * RT : (r + 1) * RT],
                            start=True,
                            stop=True,
                            perf_mode=DR,
                            skip_group_check=True,
                        )
                    dump = dpool.tile([128, 2, 512], i8, tag="dump")
                    if p == NPAIR - 1:
                        # chunk 79 is all pads: exp(0)=1.0 exactly (e5m2
                        # bits 60); constant dump plane, no fill or drain
                        nc.gpsimd.memset(dump[:, 1, :], 60)
                        src = pm[:, b0 : b0 + 512]
                        nc.scalar.activation(
                            out=dump.bitcast(f8e5)[:, 0, :], in_=src, func=AF.Exp
                        )
                    elif _PLAN[seq] == "A":
                        src = pm[:, b0 : b0 + 1024]
                        nc.scalar.activation(
                            out=dump.bitcast(f8e5), in_=src, func=AF.Exp
                        )
                    else:
                        src = pm[:, b0 : b0 + 1024]
                        nc.vector.tensor_scalar(
                            out=dump, in0=src, scalar1=A5, scalar2=B5,
                            op0=OP.mult, op1=OP.add,
                        )
                    pend.append((dump, r, p))
                    while len(pend) > 3:
                        dmp, rr, pp = pend.pop(0)
                        sc = SACCS[rr]
                        nc.tensor.matmul(
                            out=pm[:, sc : sc + 512],
                            lhsT=ones8,
                            rhs=dmp.bitcast(f8e5),
                            start=(pp == 0),
                            stop=(pp == NPAIR - 1),
                            perf_mode=DR,
                            skip_group_check=True,
                        )
                if p == 4:
                    numer_chain()
                elif p == 8:
                    nc.vector.tensor_scalar(
                        out=exp_num.bitcast(i32), in0=numer, scalar1=AEXP,
                        scalar2=BEXP, op0=OP.mult, op1=OP.add,
                    )
                    nc.vector.tensor_scalar(
                        out=exp_st.bitcast(i32), in0=tcl, scalar1=AEXP,
                        scalar2=BEXP, op0=OP.mult, op1=OP.add,
                    )
            flush_reduces()

            # ---------------- combine ----------------
            # sums for row j*128+p sit at sacc[rt(j)][0, (j%4)*128 + p]
            rowsum = singles.tile([128, NJ], f32)
            sacc_sb = singles.tile([1, 2, 512], f32)
            nc.vector.tensor_copy(
                out=sacc_sb[0:1, 0, :], in_=pm[0:1, SACCS[0] : SACCS[0] + 512]
            )
            nc.vector.tensor_copy(
                out=sacc_sb[0:1, 1, :], in_=pm[0:1, SACCS[1] : SACCS[1] + 512]
            )
            for j in range(NJ):
                eng = nc.sync if j % 2 == 0 else nc.scalar
                eng.dma_start(
                    out=rowsum[:, j : j + 1],
                    in_=sacc_sb[0:1, j // 4, (j % 4) * 128 : (j % 4) * 128 + 128],
                )
            dnum = singles.tile([128, NJ], f32)
            nc.vector.tensor_tensor(out=dnum, in0=exp_num, in1=exp_st, op=OP.subtract)
            denom = singles.tile([128, NJ], f32)
            nc.vector.scalar_tensor_tensor(
                out=denom, in0=rowsum, scalar=-float(NPAD), in1=dnum,
                op0=OP.add, op1=OP.add,
            )
            K2 = 0.3398
            ly = singles.tile([128, NJ], f32)
            nc.vector.tensor_scalar(
                out=ly, in0=denom.bitcast(i32), scalar1=1.0 / (1 << 23),
                scalar2=-127.0, op0=OP.mult, op1=OP.add,
            )
            lyi = singles.tile([128, NJ], i32)
            nc.vector.tensor_scalar(
                out=lyi, in0=ly, scalar1=1.0, scalar2=None, op0=OP.mult
            )
            lm0 = singles.tile([128, NJ], f32)
            nc.vector.tensor_tensor(out=lm0, in0=ly, in1=lyi, op=OP.subtract)
            lm = singles.tile([128, NJ], f32)
            nc.vector.scalar_tensor_tensor(
                out=lm, in0=lm0, scalar=0.0, in1=lm0, op0=OP.is_lt, op1=OP.add
            )
            lom = singles.tile([128, NJ], f32)
            nc.vector.tensor_scalar(
                out=lom, in0=lm, scalar1=-1.0, scalar2=1.0, op0=OP.mult, op1=OP.add
            )
            lq = singles.tile([128, NJ], f32)
            nc.vector.tensor_tensor(out=lq, in0=lm, in1=lom, op=OP.mult)
            la = singles.tile([128, NJ], f32)
            nc.vector.scalar_tensor_tensor(
                out=la, in0=lq, scalar=K2, in1=ly, op0=OP.mult, op1=OP.add
            )
            Lt = singles.tile([128, NJ], f32)
            nc.vector.scalar_tensor_tensor(
                out=Lt, in0=la, scalar=-math.log(2.0), in1=numer,
                op0=OP.mult, op1=OP.add,
            )
            Lrow = singles.tile([128, 1], f32)
            nc.vector.tensor_reduce(
                out=Lrow, in_=Lt, axis=mybir.AxisListType.X, op=OP.add
            )
            ones = singles.tile([128, 1], f32)
            nc.vector.memset(ones, 1.0)
            nc.tensor.matmul(
                out=pm[0:1, 0:1], lhsT=Lrow, rhs=ones, start=True, stop=True
            )
            Lp = singles.tile([1, 1], f32)
            nc.vector.tensor_copy(out=Lp, in_=pm[0:1, 0:1])
            nc.sync.dma_start(out=out_ext[:, :], in_=Lp)

    nc.finalize()
    return nc


def _get_nc():
    if "nc" not in _CACHE:
        _CACHE["nc"] = _build()
    return _CACHE["nc"]


def prepare_in_maps(x, W, target):
    import ml_dtypes

    f8 = ml_dtypes.float8_e4m3fn

    x = np.asarray(x, dtype=np.float32)
    W = np.asarray(W, dtype=np.float32)
    tgt = np.asarray(target).astype(np.int64).reshape(N)

    xn = x / np.linalg.norm(x, axis=1, keepdims=True)
    xna = (xn * np.float32(SA)).astype(np.float32)

    ws = W * np.float32(SB)
    wt = np.zeros((128, 2, CP), dtype=f8)
    wt[:, :, :C] = ws.T.reshape(2, 128, C).transpose(1, 0, 2).astype(f8)
    ts_full = np.einsum("nd,nd->n", xna, ws[tgt]).astype(np.float32)

    in_maps = []
    for c in range(NCORES):
        sl = slice(c * NLOC, (c + 1) * NLOC)
        xs = xna[sl]
        in_maps.append(
            {
                "xT": np.ascontiguousarray(
                    xs.T.reshape(2, 128, NLOC).transpose(1, 0, 2).astype(f8)
                ),
                "wt": wt,
                "ts": np.ascontiguousarray(ts_full[sl].reshape(NJ, 128).T),
            }
        )
    return in_maps


def kernel(x, W, target):
    from concourse.bass_utils import run_bass_kernel_spmd

    nc = _get_nc()
    in_maps = prepare_in_maps(x, W, target)
    res = run_bass_kernel_spmd(nc, in_maps, core_ids=list(range(NCORES)))
    parts = np.stack(
        [res.results[i]["out"].astype(np.float32).reshape(()) for i in range(NCORES)]
    )
    total = np.sum(parts, dtype=np.float32)
    return np.float32(-(total / np.float32(N)))


# revision 56
# speedup vs baseline: 1.0361x; 1.0361x over previous
"""ArcFace loss kernel v4: classes-on-partitions, PE-reduced e5m2 exp dumps.

~82-85us on 8 cores (prior baseline 110us).  Data-parallel over rows N
(each core owns 1024 rows).  Differences vs the rows-on-partitions family:
the matmul puts CLASSES on PSUM partitions (lhsT = W chunk, rhs = x rows),
the drains are single-pass (ACT exp -> fp8e5 dump with no accum_out read;
DVE e5m2-Schraudolph int8 with no reduce tree), and the per-row exp-sum
reduction over classes is done by the PE via fp8e5 DoubleRow ones-matmuls
accumulating into a per-row-tile PSUM register (partition-dim reduction is
free on the PE; the drains' free-dim direction would have needed a second
elementwise pass).  The psum drain bandwidth (ACT 1.2GHz + DVE 0.96GHz,
1 elem/cycle/lane each, ~1.9 elem/ns aggregate) is the fundamental wall;
the schedule keeps both drain engines and the (mid-clock) PE co-saturated:
  - 80 units of [128 classes, 2x512 rows] on a 3-unit/6-bank psum ring,
    strictly alternating ACT/DVE drains (unit 79 forced to ACT so the 240
    W pads contribute exactly exp(0)=1 each; the tail subtracts 240);
  - PE ones-reduces run 3+ units behind their drains so the PE instruction
    stream never blocks mid-streak (p-state stays up);
  - drains are 1024 wide (2 banks): wider instructions amortize better but
    starve the PE on a 6-bank ring.

  - W is zero-padded to 10240 classes (80 chunks of 128).  The 240 pads sit
    in the last pair, which is always ACT-drained, so each contributes
    exp(0) = 1.0 exactly; the tail subtracts 240.
  - e5m2 dumps quantize exp to 2 mantissa bits: ~5.5% rms per-element noise
    that averages out over the 10240-term row sums (constants tuned so the
    expected sum is exact; measured end error ~1e-4).
  - Target path unchanged: host supplies t_s = S*<x_n, W[tgt]> exactly; the
    device computes the margin numerator with a Quake-rsqrt chain and
    ln(denom) with a bits->log2 approximation, all on DVE.
"""

import math

import numpy as np

S = 30.0
MARGIN = 0.3
EPS = 1e-7
N, D, C = 8192, 256, 10000
NCORES = 8
NLOC = N // NCORES  # 1024 rows per core
NJ = NLOC // 128
CP = 10240  # padded class count: 80 chunks of 128
NCHUNK = CP // 128  # 80
NPAIR = NCHUNK // 2  # 40 reduce pairs
NPAD = CP - C  # 240 pad classes, exp(0)=1 each (ACT-drained)
RT = 512  # row-tile width (rhs free per matmul)
NRT = NLOC // RT  # 2 row-tiles

SA = 8.0
SB = 3.75  # SA*SB = S

# f32 Schraudolph (target-score path)
AEXP = 12102203.0
BEXP = 1064881816.0
# e5m2 Schraudolph: int8(A5*x + B5) bitcast to fp8e5 ~= exp(x)
A5 = 5.770780163555856  # 4/ln2
B5 = 59.777
RSQRT_MAGIC = 1597463007.0

# unit seq = pair*2 + rowtile; engine plan: ~52% ACT / 48% DVE, with the
# pad-carrying last pair (units 78, 79) forced to ACT
_PLAN = [
    "A" if (s % 2 == 0 or s == 79) else "D" for s in range(2 * NPAIR)
]

_CACHE = {}


def _build():
    import concourse.bass as bass  # noqa: F401
    import concourse.mybir as mybir
    import concourse.tile as tile
    from concourse import bacc

    f32 = mybir.dt.float32
    f8 = mybir.dt.float8e4
    f8e5 = mybir.dt.float8e5
    i8 = mybir.dt.int8
    i32 = mybir.dt.int32
    AF = mybir.ActivationFunctionType
    OP = mybir.AluOpType
    DR = mybir.MatmulPerfMode.DoubleRow

    nc = bacc.Bacc()
    xT_ext = nc.declare_dram_parameter("xT", [128, 2, NLOC], f8, isOutput=False)
    wt_ext = nc.declare_dram_parameter("wt", [128, 2, CP], f8, isOutput=False)
    ts_ext = nc.declare_dram_parameter("ts", [128, NJ], f32, isOutput=False)
    out_ext = nc.declare_dram_parameter("out", [1, 1], f32, isOutput=True)

    with tile.TileContext(nc) as tc:
        with (
            tc.tile_pool(name="singles", bufs=1) as singles,
            tc.tile_pool(name="dpool", bufs=12) as dpool,
            tc.tile_pool(name="pmain", bufs=1, space="PSUM") as psum_main,
        ):
            # banks 0-5: fill ring (3 units of 2 banks); banks 6/7: the two
            # per-row-tile sum accumulators (sums duplicated on all partitions)
            pm = psum_main.tile([128, 8 * 512], f32)
            SACCS = (7 * 512, 6 * 512)

            xT = singles.tile([128, 2, NLOC], f8)
            wt = singles.tile([128, 2, CP], f8)
            traw = singles.tile([128, NJ], f32)
            nc.scalar.dma_start(out=xT[:, :, 0:RT], in_=xT_ext[:, :, 0:RT])
            nc.scalar.dma_start(out=xT[:, :, RT:NLOC], in_=xT_ext[:, :, RT:NLOC])
            W_ROUNDS = [(0, 256), (256, 512), (512, 1024), (1024, 2048),
                        (2048, 4096), (4096, 6144), (6144, 8192), (8192, CP)]
            for r, (c0, c1) in enumerate(W_ROUNDS):
                eng = nc.sync if r % 2 == 0 else nc.scalar
                eng.dma_start(out=wt[:, :, c0:c1], in_=wt_ext[:, :, c0:c1])
            nc.scalar.dma_start(out=traw, in_=ts_ext[:, :])

            ones8 = singles.tile([128, 2, 128], f8e5)
            nc.vector.memset(ones8, 1.0)
            # preload the ACT Exp table while the W DMA streams (else the
            # first real drain pays the 1.3us ACT_TABLE_LOAD)
            tdump = singles.tile([128, 32], f32)
            nc.scalar.activation(
                out=tdump, in_=ones8.bitcast(f32)[:, 0:1], func=AF.Exp
            )

            rs_seed = singles.tile([128, NJ], i32)
            rs_t1 = singles.tile([128, NJ], f32)
            rs_y1 = singles.tile([128, NJ], f32)
            rs_t2 = singles.tile([128, NJ], f32)

            def rsqrt2(src, dst, fold=1.0):
                nc.vector.tensor_scalar(
                    out=rs_seed, in0=src.bitcast(i32), scalar1=-0.5,
                    scalar2=RSQRT_MAGIC, op0=OP.mult, op1=OP.add,
                )
                y0 = rs_seed.bitcast(f32)
                nc.vector.tensor_tensor(out=rs_t1, in0=y0, in1=y0, op=OP.mult)
                nc.vector.tensor_tensor(out=rs_t1, in0=rs_t1, in1=src, op=OP.mult)
                nc.vector.tensor_scalar(
                    out=rs_t1, in0=rs_t1, scalar1=-0.5, scalar2=1.5,
                    op0=OP.mult, op1=OP.add,
                )
                nc.vector.tensor_tensor(out=rs_y1, in0=y0, in1=rs_t1, op=OP.mult)
                nc.vector.tensor_tensor(out=rs_t2, in0=rs_y1, in1=rs_y1, op=OP.mult)
                nc.vector.tensor_tensor(out=rs_t2, in0=rs_t2, in1=src, op=OP.mult)
                nc.vector.tensor_scalar(
                    out=rs_t2, in0=rs_t2, scalar1=-0.5 * fold, scalar2=1.5 * fold,
                    op0=OP.mult, op1=OP.add,
                )
                nc.vector.tensor_tensor(out=dst, in0=rs_y1, in1=rs_t2, op=OP.mult)

            tcl = singles.tile([128, NJ], f32)
            usq = singles.tile([128, NJ], f32)
            rsu = singles.tile([128, NJ], f32)
            rtm = singles.tile([128, NJ], f32)
            numer = singles.tile([128, NJ], f32)
            exp_num = singles.tile([128, NJ], f32)
            exp_st = singles.tile([128, NJ], f32)

            def numer_chain():
                sclip = S * (1.0 - EPS)
                nc.vector.tensor_scalar(
                    out=tcl, in0=traw, scalar1=-sclip, scalar2=sclip,
                    op0=OP.max, op1=OP.min,
                )
                nc.vector.tensor_tensor(out=usq, in0=tcl, in1=tcl, op=OP.mult)
                nc.vector.tensor_scalar(
                    out=usq, in0=usq, scalar1=-1.0, scalar2=S * S,
                    op0=OP.mult, op1=OP.add,
                )
                rsqrt2(usq, rsu, fold=-math.sin(MARGIN))
                nc.vector.tensor_tensor(out=rtm, in0=usq, in1=rsu, op=OP.mult)
                nc.vector.scalar_tensor_tensor(
                    out=numer, in0=tcl, scalar=math.cos(MARGIN), in1=rtm,
                    op0=OP.mult, op1=OP.add,
                )

            # ---------------- main loop: 80 units of [128cls, 2, 512rows] --
            pend = []  # (dump, rowtile, pair) awaiting their PE reduce

            def flush_reduces():
                while pend:
                    dmp, r, p = pend.pop(0)
                    sc = SACCS[r]
                    nc.tensor.matmul(
                        out=pm[:, sc : sc + 512],
                        lhsT=ones8,
                        rhs=dmp.bitcast(f8e5),
                        start=(p == 0),
                        stop=(p == NPAIR - 1),
                        perf_mode=DR,
                        skip_group_check=True,
                    )

            for p in range(NPAIR):
                for r in range(NRT):
                    seq = p * 2 + r
                    b0 = (seq % 3) * 1024
                    for gg in range(2):
                        g = 2 * p + gg
                        nc.tensor.matmul(
                            out=pm[:, b0 + gg * 512 : b0 + (gg + 1) * 512],
                            lhsT=wt[:, :, g * 128 : (g + 1) * 128],
                            rhs=xT[:, :, r * RT : (r + 1) * RT],
                            start=True,
                            stop=True,
                            perf_mode=DR,
                            skip_group_check=True,
                        )
                    dump = dpool.tile([128, 2, 512], i8, tag="dump")
                    src = pm[:, b0 : b0 + 1024]
                    if _PLAN[seq] == "A":
                        nc.scalar.activation(
                            out=dump.bitcast(f8e5), in_=src, func=AF.Exp
                        )
                    else:
                        nc.vector.tensor_scalar(
                            out=dump, in0=src, scalar1=A5, scalar2=B5,
                            op0=OP.mult, op1=OP.add,
                        )
                    pend.append((dump, r, p))
                    while len(pend) > 3:
                        dmp, rr, pp = pend.pop(0)
                        sc = SACCS[rr]
                        nc.tensor.matmul(
                            out=pm[:, sc : sc + 512],
                            lhsT=ones8,
                            rhs=dmp.bitcast(f8e5),
                            start=(pp == 0),
                            stop=(pp == NPAIR - 1),
                            perf_mode=DR,
                            skip_group_check=True,
                        )
                if p == 4:
                    numer_chain()
                elif p == 8:
                    nc.vector.tensor_scalar(
                        out=exp_num.bitcast(i32), in0=numer, scalar1=AEXP,
                        scalar2=BEXP, op0=OP.mult, op1=OP.add,
                    )
                    nc.vector.tensor_scalar(
                        out=exp_st.bitcast(i32), in0=tcl, scalar1=AEXP,
                        scalar2=BEXP, op0=OP.mult, op1=OP.add,
                    )
            flush_reduces()

            # ---------------- combine ----------------
            # sums for row j*128+p sit at sacc[rt(j)][0, (j%4)*128 + p]
            rowsum = singles.tile([128, NJ], f32)
            sacc_sb = singles.tile([1, 2, 512], f32)
            nc.vector.tensor_copy(
                out=sacc_sb[0:1, 0, :], in_=pm[0:1, SACCS[0] : SACCS[0] + 512]
            )
            nc.vector.tensor_copy(
                out=sacc_sb[0:1, 1, :], in_=pm[0:1, SACCS[1] : SACCS[1] + 512]
            )
            for j in range(NJ):
                eng = nc.sync if j % 2 == 0 else nc.scalar
                eng.dma_start(
                    out=rowsum[:, j : j + 1],
                    in_=sacc_sb[0:1, j // 4, (j % 4) * 128 : (j % 4) * 128 + 128],
                )
            dnum = singles.tile([128, NJ], f32)
            nc.vector.tensor_tensor(out=dnum, in0=exp_num, in1=exp_st, op=OP.subtract)
            denom = singles.tile([128, NJ], f32)
            nc.vector.scalar_tensor_tensor(
                out=denom, in0=rowsum, scalar=-float(NPAD), in1=dnum,
                op0=OP.add, op1=OP.add,
            )
            K2 = 0.3398
            ly = singles.tile([128, NJ], f32)
            nc.vector.tensor_scalar(
                out=ly, in0=denom.bitcast(i32), scalar1=1.0 / (1 << 23),
                scalar2=-127.0, op0=OP.mult, op1=OP.add,
            )
            lyi = singles.tile([128, NJ], i32)
            nc.vector.tensor_scalar(
                out=lyi, in0=ly, scalar1=1.0, scalar2=None, op0=OP.mult
            )
            lm0 = singles.tile([128, NJ], f32)
            nc.vector.tensor_tensor(out=lm0, in0=ly, in1=lyi, op=OP.subtract)
            lm = singles.tile([128, NJ], f32)
            nc.vector.scalar_tensor_tensor(
                out=lm, in0=lm0, scalar=0.0, in1=lm0, op0=OP.is_lt, op1=OP.add
            )
            lom = singles.tile([128, NJ], f32)
            nc.vector.tensor_scalar(
                out=lom, in0=lm, scalar1=-1.0, scalar2=1.0, op0=OP.mult, op1=OP.add
            )
            lq = singles.tile([128, NJ], f32)
            nc.vector.tensor_tensor(out=lq, in0=lm, in1=lom, op=OP.mult)
            la = singles.tile([128, NJ], f32)
            nc.vector.scalar_tensor_tensor(
                out=la, in0=lq, scalar=K2, in1=ly, op0=OP.mult, op1=OP.add
            )
            Lt = singles.tile([128, NJ], f32)
            nc.vector.scalar_tensor_tensor(
                out=Lt, in0=la, scalar=-math.log(2.0), in1=numer,
                op0=OP.mult, op1=OP.add,
            )
            Lrow = singles.tile([128, 1], f32)
            nc.vector.tensor_reduce(
                out=Lrow, in_=Lt, axis=mybir.AxisListType.X, op=OP.add
            )
            ones = singles.tile([128, 1], f32)
            nc.vector.memset(ones, 1.0)
            nc.tensor.matmul(
                out=pm[0:1, 0:1], lhsT=Lrow, rhs=ones, start=True, stop=True
            )
            Lp = singles.tile([1, 1], f32)
            nc.vector.tensor_copy(out=Lp, in_=pm[0:1, 0:1])
            nc.sync.dma_start(out=out_ext[:, :], in_=Lp)

    nc.finalize()
    return nc


def _get_nc():
    if "nc" not in _CACHE:
        _CACHE["nc"] = _build()
    return _CACHE["nc"]


def prepare_in_maps(x, W, target):
    import ml_dtypes

    f8 = ml_dtypes.float8_e4m3fn

    x = np.asarray(x, dtype=np.float32)
    W = np.asarray(W, dtype=np.float32)
    tgt = np.asarray(target).astype(np.int64).reshape(N)

    xn = x / np.linalg.norm(x, axis=1, keepdims=True)
    xna = (xn * np.float32(SA)).astype(np.float32)

    ws = W * np.float32(SB)
    wt = np.zeros((128, 2, CP), dtype=f8)
    wt[:, :, :C] = ws.T.reshape(2, 128, C).transpose(1, 0, 2).astype(f8)
    ts_full = np.einsum("nd,nd->n", xna, ws[tgt]).astype(np.float32)

    in_maps = []
    for c in range(NCORES):
        sl = slice(c * NLOC, (c + 1) * NLOC)
        xs = xna[sl]
        in_maps.append(
            {
                "xT": np.ascontiguousarray(
                    xs.T.reshape(2, 128, NLOC).transpose(1, 0, 2).astype(f8)
                ),
                "wt": wt,
                "ts": np.ascontiguousarray(ts_full[sl].reshape(NJ, 128).T),
            }
        )
    return in_maps


def kernel(x, W, target):
    from concourse.bass_utils import run_bass_kernel_spmd

    nc = _get_nc()
    in_maps = prepare_in_maps(x, W, target)
    res = run_bass_kernel_spmd(nc, in_maps, core_ids=list(range(NCORES)))
    parts = np.stack(
        [res.results[i]["out"].astype(np.float32).reshape(()) for i in range(NCORES)]
    )
    total = np.sum(parts, dtype=np.float32)
    return np.float32(-(total / np.float32(N)))


# revision 58
# speedup vs baseline: 1.0409x; 1.0046x over previous
"""ArcFace loss kernel v4: classes-on-partitions, PE-reduced e5m2 exp dumps.

~82-85us on 8 cores (prior baseline 110us).  Data-parallel over rows N
(each core owns 1024 rows).  Differences vs the rows-on-partitions family:
the matmul puts CLASSES on PSUM partitions (lhsT = W chunk, rhs = x rows),
the drains are single-pass (ACT exp -> fp8e5 dump with no accum_out read;
DVE e5m2-Schraudolph int8 with no reduce tree), and the per-row exp-sum
reduction over classes is done by the PE via fp8e5 DoubleRow ones-matmuls
accumulating into a per-row-tile PSUM register (partition-dim reduction is
free on the PE; the drains' free-dim direction would have needed a second
elementwise pass).  The psum drain bandwidth (ACT 1.2GHz + DVE 0.96GHz,
1 elem/cycle/lane each, ~1.9 elem/ns aggregate) is the fundamental wall;
the schedule keeps both drain engines and the (mid-clock) PE co-saturated:
  - 80 units of [128 classes, 2x512 rows] on a 3-unit/6-bank psum ring,
    strictly alternating ACT/DVE drains (unit 79 forced to ACT so the 240
    W pads contribute exactly exp(0)=1 each; the tail subtracts 240);
  - PE ones-reduces run 3+ units behind their drains so the PE instruction
    stream never blocks mid-streak (p-state stays up);
  - drains are 1024 wide (2 banks): wider instructions amortize better but
    starve the PE on a 6-bank ring.

  - W is zero-padded to 10240 classes (80 chunks of 128).  The 240 pads sit
    in the last pair, which is always ACT-drained, so each contributes
    exp(0) = 1.0 exactly; the tail subtracts 240.
  - e5m2 dumps quantize exp to 2 mantissa bits: ~5.5% rms per-element noise
    that averages out over the 10240-term row sums (constants tuned so the
    expected sum is exact; measured end error ~1e-4).
  - Target path unchanged: host supplies t_s = S*<x_n, W[tgt]> exactly; the
    device computes the margin numerator with a Quake-rsqrt chain and
    ln(denom) with a bits->log2 approximation, all on DVE.
"""

import math

import numpy as np

S = 30.0
MARGIN = 0.3
EPS = 1e-7
N, D, C = 8192, 256, 10000
NCORES = 8
NLOC = N // NCORES  # 1024 rows per core
NJ = NLOC // 128
CP = 10240  # padded class count: 80 chunks of 128
NCHUNK = CP // 128  # 80
NPAIR = NCHUNK // 2  # 40 reduce pairs
NPAD = CP - C  # 240 pad classes, exp(0)=1 each (ACT-drained)
RT = 512  # row-tile width (rhs free per matmul)
NRT = NLOC // RT  # 2 row-tiles

SA = 8.0
SB = 3.75  # SA*SB = S

# f32 Schraudolph (target-score path)
AEXP = 12102203.0
BEXP = 1064881816.0
# e5m2 Schraudolph: int8(A5*x + B5) bitcast to fp8e5 ~= exp(x)
A5 = 5.770780163555856  # 4/ln2
B5 = 59.777
RSQRT_MAGIC = 1597463007.0

# unit seq = pair*2 + rowtile; engine plan: ~52% ACT / 48% DVE, with the
# pad-carrying last pair (units 78, 79) forced to ACT
_PLAN = [
    "A" if (s % 2 == 0 or s == 79) else "D" for s in range(2 * NPAIR)
]

_CACHE = {}


def _build():
    import concourse.bass as bass  # noqa: F401
    import concourse.mybir as mybir
    import concourse.tile as tile
    from concourse import bacc

    f32 = mybir.dt.float32
    f8 = mybir.dt.float8e4
    f8e5 = mybir.dt.float8e5
    i8 = mybir.dt.int8
    i32 = mybir.dt.int32
    AF = mybir.ActivationFunctionType
    OP = mybir.AluOpType
    DR = mybir.MatmulPerfMode.DoubleRow

    nc = bacc.Bacc()
    xT_ext = nc.declare_dram_parameter("xT", [128, 2, NLOC], f8, isOutput=False)
    wt_ext = nc.declare_dram_parameter("wt", [128, 2, CP], f8, isOutput=False)
    ts_ext = nc.declare_dram_parameter("ts", [128, NJ], f32, isOutput=False)
    out_ext = nc.declare_dram_parameter("out", [1, 1], f32, isOutput=True)

    with tile.TileContext(nc) as tc:
        with (
            tc.tile_pool(name="singles", bufs=1) as singles,
            tc.tile_pool(name="dpool", bufs=12) as dpool,
            tc.tile_pool(name="pmain", bufs=1, space="PSUM") as psum_main,
        ):
            # banks 0-5: fill ring (3 units of 2 banks); banks 6/7: the two
            # per-row-tile sum accumulators (sums duplicated on all partitions)
            pm = psum_main.tile([128, 8 * 512], f32)
            SACCS = (7 * 512, 6 * 512)

            xT = singles.tile([128, 2, NLOC], f8)
            wt = singles.tile([128, 2, CP], f8)
            traw = singles.tile([128, NJ], f32)
            nc.scalar.dma_start(out=xT[:, :, 0:RT], in_=xT_ext[:, :, 0:RT])
            nc.scalar.dma_start(out=xT[:, :, RT:NLOC], in_=xT_ext[:, :, RT:NLOC])
            W_ROUNDS = [(0, 256), (256, 512), (512, 1024), (1024, 2048),
                        (2048, 4096), (4096, 6144), (6144, 8192), (8192, CP)]
            for r, (c0, c1) in enumerate(W_ROUNDS):
                eng = nc.sync if r % 2 == 0 else nc.scalar
                eng.dma_start(out=wt[:, :, c0:c1], in_=wt_ext[:, :, c0:c1])
            nc.scalar.dma_start(out=traw, in_=ts_ext[:, :])

            ones8 = singles.tile([128, 2, 128], f8e5)
            nc.vector.memset(ones8, 1.0)
            # preload the ACT Exp table while the W DMA streams (else the
            # first real drain pays the 1.3us ACT_TABLE_LOAD)
            tdump = singles.tile([128, 32], f32)
            nc.scalar.activation(
                out=tdump, in_=ones8.bitcast(f32)[:, 0:1], func=AF.Exp
            )

            rs_seed = singles.tile([128, NJ], i32)
            rs_t1 = singles.tile([128, NJ], f32)
            rs_y1 = singles.tile([128, NJ], f32)
            rs_t2 = singles.tile([128, NJ], f32)

            def rsqrt2(src, dst, fold=1.0):
                nc.vector.tensor_scalar(
                    out=rs_seed, in0=src.bitcast(i32), scalar1=-0.5,
                    scalar2=RSQRT_MAGIC, op0=OP.mult, op1=OP.add,
                )
                y0 = rs_seed.bitcast(f32)
                nc.vector.tensor_tensor(out=rs_t1, in0=y0, in1=y0, op=OP.mult)
                nc.vector.tensor_tensor(out=rs_t1, in0=rs_t1, in1=src, op=OP.mult)
                nc.vector.tensor_scalar(
                    out=rs_t1, in0=rs_t1, scalar1=-0.5, scalar2=1.5,
                    op0=OP.mult, op1=OP.add,
                )
                nc.vector.tensor_tensor(out=rs_y1, in0=y0, in1=rs_t1, op=OP.mult)
                nc.vector.tensor_tensor(out=rs_t2, in0=rs_y1, in1=rs_y1, op=OP.mult)
                nc.vector.tensor_tensor(out=rs_t2, in0=rs_t2, in1=src, op=OP.mult)
                nc.vector.tensor_scalar(
                    out=rs_t2, in0=rs_t2, scalar1=-0.5 * fold, scalar2=1.5 * fold,
                    op0=OP.mult, op1=OP.add,
                )
                nc.vector.tensor_tensor(out=dst, in0=rs_y1, in1=rs_t2, op=OP.mult)

            tcl = singles.tile([128, NJ], f32)
            usq = singles.tile([128, NJ], f32)
            rsu = singles.tile([128, NJ], f32)
            rtm = singles.tile([128, NJ], f32)
            numer = singles.tile([128, NJ], f32)
            exp_num = singles.tile([128, NJ], f32)
            exp_st = singles.tile([128, NJ], f32)

            def numer_chain():
                sclip = S * (1.0 - EPS)
                nc.vector.tensor_scalar(
                    out=tcl, in0=traw, scalar1=-sclip, scalar2=sclip,
                    op0=OP.max, op1=OP.min,
                )
                nc.vector.tensor_tensor(out=usq, in0=tcl, in1=tcl, op=OP.mult)
                nc.vector.tensor_scalar(
                    out=usq, in0=usq, scalar1=-1.0, scalar2=S * S,
                    op0=OP.mult, op1=OP.add,
                )
                rsqrt2(usq, rsu, fold=-math.sin(MARGIN))
                nc.vector.tensor_tensor(out=rtm, in0=usq, in1=rsu, op=OP.mult)
                nc.vector.scalar_tensor_tensor(
                    out=numer, in0=tcl, scalar=math.cos(MARGIN), in1=rtm,
                    op0=OP.mult, op1=OP.add,
                )

            # ---------------- main loop: 80 units of [128cls, 2, 512rows] --
            pend = []  # (dump, rowtile, pair) awaiting their PE reduce

            def flush_reduces():
                while pend:
                    dmp, r, p = pend.pop(0)
                    sc = SACCS[r]
                    nc.tensor.matmul(
                        out=pm[:, sc : sc + 512],
                        lhsT=ones8,
                        rhs=dmp.bitcast(f8e5),
                        start=(p == 0),
                        stop=(p == NPAIR - 1),
                        perf_mode=DR,
                        skip_group_check=True,
                    )

            for p in range(NPAIR):
                for r in range(NRT):
                    seq = p * 2 + r
                    b0 = (seq % 3) * 1024
                    for gg in range(2):
                        g = 2 * p + gg
                        nc.tensor.matmul(
                            out=pm[:, b0 + gg * 512 : b0 + (gg + 1) * 512],
                            lhsT=wt[:, :, g * 128 : (g + 1) * 128],
                            rhs=xT[:, :, r * RT : (r + 1) * RT],
                            start=True,
                            stop=True,
                            perf_mode=DR,
                            skip_group_check=True,
                        )
                    dump = dpool.tile([128, 2, 512], i8, tag="dump")
                    src = pm[:, b0 : b0 + 1024]
                    if _PLAN[seq] == "A":
                        nc.scalar.activation(
                            out=dump.bitcast(f8e5), in_=src, func=AF.Exp
                        )
                    else:
                        nc.vector.tensor_scalar(
                            out=dump, in0=src, scalar1=A5, scalar2=B5,
                            op0=OP.mult, op1=OP.add,
                        )
                    pend.append((dump, r, p))
                    while len(pend) > 3:
                        dmp, rr, pp = pend.pop(0)
                        sc = SACCS[rr]
                        nc.tensor.matmul(
                            out=pm[:, sc : sc + 512],
                            lhsT=ones8,
                            rhs=dmp.bitcast(f8e5),
                            start=(pp == 0),
                            stop=(pp == NPAIR - 1),
                            perf_mode=DR,
                            skip_group_check=True,
                        )
                if p == 4:
                    numer_chain()
                elif p == 8:
                    nc.vector.tensor_scalar(
                        out=exp_num.bitcast(i32), in0=numer, scalar1=AEXP,
                        scalar2=BEXP, op0=OP.mult, op1=OP.add,
                    )
                    nc.vector.tensor_scalar(
                        out=exp_st.bitcast(i32), in0=tcl, scalar1=AEXP,
                        scalar2=BEXP, op0=OP.mult, op1=OP.add,
                    )
            flush_reduces()

            # ---------------- combine ----------------
            # sums for row j*128+p sit at sacc[rt(j)][0, (j%4)*128 + p]
            rowsum = singles.tile([128, NJ], f32)
            sacc_sb = singles.tile([1, 2, 512], f32)
            nc.vector.tensor_copy(
                out=sacc_sb[0:1, 0, :], in_=pm[0:1, SACCS[0] : SACCS[0] + 512]
            )
            nc.vector.tensor_copy(
                out=sacc_sb[0:1, 1, :], in_=pm[0:1, SACCS[1] : SACCS[1] + 512]
            )
            for j in range(NJ):
                eng = nc.sync if j % 2 == 0 else nc.scalar
                eng.dma_start(
                    out=rowsum[:, j : j + 1],
                    in_=sacc_sb[0:1, j // 4, (j % 4) * 128 : (j % 4) * 128 + 128],
                )
            dnum = singles.tile([128, NJ], f32)
            nc.vector.tensor_tensor(out=dnum, in0=exp_num, in1=exp_st, op=OP.subtract)
            denom = singles.tile([128, NJ], f32)
            nc.vector.scalar_tensor_tensor(
                out=denom, in0=rowsum, scalar=-float(NPAD), in1=dnum,
                op0=OP.add, op1=OP.add,
            )
            K2 = 0.3398
            ly = singles.tile([128, NJ], f32)
            nc.vector.tensor_scalar(
                out=ly, in0=denom.bitcast(i32), scalar1=1.0 / (1 << 23),
                scalar2=-127.0, op0=OP.mult, op1=OP.add,
            )
            lyi = singles.tile([128, NJ], i32)
            nc.vector.tensor_scalar(
                out=lyi, in0=ly, scalar1=1.0, scalar2=None, op0=OP.mult
            )
            lm0 = singles.tile([128, NJ], f32)
            nc.vector.tensor_tensor(out=lm0, in0=ly, in1=lyi, op=OP.subtract)
            lm = singles.tile([128, NJ], f32)
            nc.vector.scalar_tensor_tensor(
                out=lm, in0=lm0, scalar=0.0, in1=lm0, op0=OP.is_lt, op1=OP.add
            )
            lom = singles.tile([128, NJ], f32)
            nc.vector.tensor_scalar(
                out=lom, in0=lm, scalar1=-1.0, scalar2=1.0, op0=OP.mult, op1=OP.add
            )
            lq = singles.tile([128, NJ], f32)
            nc.vector.tensor_tensor(out=lq, in0=lm, in1=lom, op=OP.mult)
            la = singles.tile([128, NJ], f32)
            nc.vector.scalar_tensor_tensor(
                out=la, in0=lq, scalar=K2, in1=ly, op0=OP.mult, op1=OP.add
            )
            Lt = singles.tile([128, NJ], f32)
            nc.vector.scalar_tensor_tensor(
                out=Lt, in0=la, scalar=-math.log(2.0), in1=numer,
                op0=OP.mult, op1=OP.add,
            )
            Lrow = singles.tile([128, 1], f32)
            nc.vector.tensor_reduce(
                out=Lrow, in_=Lt, axis=mybir.AxisListType.X, op=OP.add
            )
            ones = singles.tile([128, 1], f32)
            nc.vector.memset(ones, 1.0)
            nc.tensor.matmul(
                out=pm[0:1, 0:1], lhsT=Lrow, rhs=ones, start=True, stop=True
            )
            Lp = singles.tile([1, 1], f32)
            nc.vector.tensor_copy(out=Lp, in_=pm[0:1, 0:1])
            nc.sync.dma_start(out=out_ext[:, :], in_=Lp)

    nc.finalize()
    return nc


def _get_nc():
    if "nc" not in _CACHE:
        _CACHE["nc"] = _build()
    return _CACHE["nc"]


def prepare_in_maps(x, W, target):
    import ml_dtypes

    f8 = ml_dtypes.float8_e4m3fn

    x = np.asarray(x, dtype=np.float32)
    W = np.asarray(W, dtype=np.float32)
    tgt = np.asarray(target).astype(np.int64).reshape(N)

    xn = x / np.linalg.norm(x, axis=1, keepdims=True)
    xna = (xn * np.float32(SA)).astype(np.float32)

    ws = W * np.float32(SB)
    wt = np.zeros((128, 2, CP), dtype=f8)
    wt[:, :, :C] = ws.T.reshape(2, 128, C).transpose(1, 0, 2).astype(f8)
    ts_full = np.einsum("nd,nd->n", xna, ws[tgt]).astype(np.float32)

    in_maps = []
    for c in range(NCORES):
        sl = slice(c * NLOC, (c + 1) * NLOC)
        xs = xna[sl]
        in_maps.append(
            {
                "xT": np.ascontiguousarray(
                    xs.T.reshape(2, 128, NLOC).transpose(1, 0, 2).astype(f8)
                ),
                "wt": wt,
                "ts": np.ascontiguousarray(ts_full[sl].reshape(NJ, 128).T),
            }
        )
    return in_maps


def kernel(x, W, target):
    from concourse.bass_utils import run_bass_kernel_spmd

    nc = _get_nc()
    in_maps = prepare_in_maps(x, W, target)
    res = run_bass_kernel_spmd(nc, in_maps, core_ids=list(range(NCORES)))
    parts = np.stack(
        [res.results[i]["out"].astype(np.float32).reshape(()) for i in range(NCORES)]
    )
    total = np.sum(parts, dtype=np.float32)
    return np.float32(-(total / np.float32(N)))
